# revision 2
# baseline (speedup 1.0000x reference)
"""Multi-head attention (B=4, S=2048, E=768, H=12) on 8 trn2 NeuronCores.

Sharding: tensor-parallel over heads x data-parallel over batch. Core c
handles batch b=c//2 and heads 6*(c%2)..6*(c%2)+5 (all 2048 queries). Each
core emits a partial output projection (its 6 heads' contribution); the two
cores of a batch pair are summed on the host during unsharding, where the
bias is also added (host-side add is part of the same unshard pass that
already sums the core pairs).

v2 design (from the v1 trace: PE busy 285.6us/322us = the bottleneck, with
Scalar exp at 226.6us right behind):
  - x^T comes from 24 xbar transpose-DMAs (dma_start_transpose) straight
    out of DRAM fp16 -- the 96 PE transposes and 16 ScalarE casts of v1
    are gone (-26us PE, -15us ACT). Host supplies x as fp16.
  - S^T matmuls contract only d=64, so the PE array is half idle. The two
    heads of an f-tile live at partition bases 0 and 64, so their S^T
    matmuls go to disjoint row-groups and run CONCURRENTLY (row tiling,
    tile_position auto-derived from base partitions): S^T 82us -> ~43us.
  - The exp stream (25.2M logits/core) is split across two engines: ACT
    runs exact exp (9/16 of tiles); DVE runs a Schraudolph bit-trick exp
    (7/16): one tensor_scalar (mult,add) producing int16 fp16-bits
    (numpy sim of the full pipeline: rel err 1.0e-2 vs the 2e-2 budget).
    The softmax denominator is the V ones-column, so numerator and
    denominator use identical approximated weights and the residual error
    is a mild reweighting.
  - Stage B runs in 12 blocks (4 query-chunks x 3 head-pairs) of 8
    k-tile-pairs. PSUM: 3-deep [128,1024] S ring (6 banks) + 2 AV
    accumulator banks = 8 exactly. AV accumulates all 16 k-tiles per
    (head, q-chunk) with DV=65 (ones column) at LAG>=2 behind exp.
  - Softmax normalization multiplies run on the otherwise idle GpSimd
    engine; the reciprocal keeps v1's DRAM-bounce partition broadcast.
  - Output projection is woven per query-chunk, reusing the AV psum
    slots; results leave as fp16 and are upcast host-side.

Environment workarounds (this walrus build): sync-waits are split one per
instruction onto NoOps (_split_waits, _TC).
"""

import math

import numpy as np

import concourse.bass as bass
import concourse.tile as tile
from concourse import mybir
from concourse.bass_utils import run_bass_kernel_spmd
from concourse.tile import ScopedClock

B, S, E, H, D = 4, 2048, 768, 12, 64
NCORES = 8
HL = 6               # heads per core
FL = HL * D          # 384 local feature dim
SCALE = D ** -0.5
FP = mybir.dt.float32
F16 = mybir.dt.float16
I16 = mybir.dt.int16
P = 128

ET = E // P          # 6 e-chunks of 128
FT = FL // P         # 3 local f-tiles of 128
NKT = S // P         # 16 k-tiles of 128
NQC = S // 512       # 4 query chunks of 512
NST = S // P         # 16 s-tiles
DV = D + 1           # 65: V plus ones column

# Schraudolph fp16-bit exp: bits = round(SCALE*s*1024/ln2 + 15360 + SIGMA)
SIGMA = -60.0
K1S = SCALE * 1024.0 / math.log(2.0)
K2S = 15360.0 + SIGMA
# ACT gets 9 of every 16 exp tiles (ACT ~1147ns vs DVE ~1192ns + DVE extras)
ACT_SHARE = 9

LAG = 2              # AV trails exp by this many k-tile pairs


class _TC(tile.TileContext):
    """TileContext with the end-of-kernel drain's sem waits split one per
    instruction (this walrus build's CTRL_NO_STRUCT encoding holds only one
    sync wait; the stock drain carries one wait per outstanding proc)."""

    def _drain_and_barrier(self, tick_clock, wait_clock):
        probe = self.nc.sync.nop()
        wait_clock.add_sem_waits(
            probe.ins, ScopedClock({None: tick_clock.global_clock})
        )
        si = probe.ins.sync_info
        waits = list(si.on_wait) if si is not None else []
        if len(waits) > 1:
            si.on_wait = waits[:1]
            for w in waits[1:]:
                n = self.nc.sync.nop()
                n.ins.sync_info = type(si)(on_wait=[w], on_update=[])
        self.nc.sync.drain()
        self.nc.all_engine_barrier()
        popped = self.nc._tile_sem_poison_stack.pop()
        assert popped is self._sem_poison
        self.nc.clear_and_free_semaphores(list(self.sems.allocated().values()))
        self.nc.all_engine_barrier()


def _split_waits(nc):
    """This walrus build accepts at most one sync-wait per TPB instruction
    (two on EventSemaphore). Tile emits up to 2-3. Hoist the extras onto
    same-engine NoOps inserted immediately before the instruction."""
    ctr = [0]
    for f in nc.m.functions:
        for bb in f.blocks:
            out = []
            changed = False
            for inst in bb.instructions:
                si = getattr(inst, "sync_info", None)
                if si is not None and si.on_wait:
                    cap = 2 if isinstance(inst, mybir.InstEventSemaphore) else 1
                    waits = list(si.on_wait)
                    if len(waits) > cap:
                        changed = True
                        for w in waits[:-cap]:
                            ctr[0] += 1
                            out.append(
                                mybir.InstNoOp(
                                    name=f"WSPLIT-{ctr[0]}",
                                    engine=inst.engine,
                                    ins=[],
                                    outs=[],
                                    sync_info=mybir.SyncInfo(
                                        on_wait=[w], on_update=[]
                                    ),
                                    bass_nofuse=True,
                                )
                            )
                        si.on_wait = waits[-cap:]
                        inst.sync_info = si
                out.append(inst)
            if changed:
                bb.instructions = out


def build():
    nc = bass.Bass()
    xb = nc.dram_tensor("xb", [S, E], F16, kind="ExternalInput")
    wqkvT = nc.dram_tensor("wqkvT", [E, 3 * FL], F16, kind="ExternalInput")
    wprojT = nc.dram_tensor("wprojT", [FL, E], F16, kind="ExternalInput")
    out16 = nc.dram_tensor("out16", [S, E], F16, kind="ExternalOutput")

    Exp = mybir.ActivationFunctionType.Exp
    Copy = mybir.ActivationFunctionType.Copy
    Mult = mybir.AluOpType.mult
    Add = mybir.AluOpType.add

    from contextlib import ExitStack

    with _TC(nc) as tc, ExitStack() as stack:
        consts = stack.enter_context(tc.tile_pool(name="consts", bufs=1))
        persist = stack.enter_context(tc.tile_pool(name="persist", bufs=1))

        wqkv_sb = [
            consts.tile([P, 3 * FL], F16, tag=f"wqkv{c}", name=f"wqkv{c}")
            for c in range(ET)
        ]
        wproj_sb = [
            consts.tile([P, E], F16, tag=f"wproj{c}", name=f"wproj{c}")
            for c in range(FT)
        ]
        xbT = [
            persist.tile([P, S], F16, tag=f"xbT{c}", name=f"xbT{c}")
            for c in range(ET)
        ]
        qT = [persist.tile([P, S], F16, tag=f"qT{t}", name=f"qT{t}") for t in range(FT)]
        kT = [persist.tile([P, S], F16, tag=f"kT{t}", name=f"kT{t}") for t in range(FT)]
        vp = [persist.tile([P, HL * DV], F16, tag=f"vp{t}", name=f"vp{t}") for t in range(NST)]
        outT = [persist.tile([P, S], F16, tag=f"outT{t}", name=f"outT{t}") for t in range(FT)]

        # ---------------- input DMAs ----------------
        for c in range(ET):
            nc.scalar.dma_start(wqkv_sb[c][:], wqkvT[P * c : P * (c + 1), :])
        for c in range(FT):
            nc.scalar.dma_start(wproj_sb[c][:], wprojT[P * c : P * (c + 1), :])
        # x^T via xbar transpose DMAs, j-chunk outer so early consumers
        # (which need all e-chunks of one 512-column group) unblock first
        for j in range(4):
            for c in range(ET):
                eng = nc.sync if (c % 2 == 0) else nc.scalar
                eng.dma_start_transpose(
                    xbT[c][:, 512 * j : 512 * (j + 1)],
                    xb[512 * j : 512 * (j + 1), P * c : P * (c + 1)],
                )

        # ---------------- Stage A: QKV projections ----------------
        with tc.tile_pool(name="mm_psum", bufs=3, space="PSUM") as mm_psum:

            def qk_chunk(which, ft, j):
                dst = qT if which == 0 else kT
                pq = mm_psum.tile([P, 512], FP, tag="mm", name=f"pq{which}_{ft}_{j}")
                for c in range(ET):
                    nc.tensor.matmul(
                        pq[:],
                        wqkv_sb[c][:, FL * which + P * ft : FL * which + P * (ft + 1)],
                        xbT[c][:, 512 * j : 512 * (j + 1)],
                        start=(c == 0),
                        stop=(c == ET - 1),
                    )
                nc.vector.tensor_copy(dst[ft][:, 512 * j : 512 * (j + 1)], pq[:])

            def v_tile(t):
                pv = mm_psum.tile([P, 512], FP, tag="mm", name=f"pv{t}")
                for c in range(ET):
                    nc.tensor.matmul(
                        pv[:, :FL],
                        xbT[c][:, P * t : P * (t + 1)],
                        wqkv_sb[c][:, 2 * FL : 3 * FL],
                        start=(c == 0),
                        stop=(c == ET - 1),
                    )
                v3 = vp[t].rearrange("p (h d) -> p h d", d=DV)
                nc.vector.tensor_copy(
                    v3[:, :, 0:D], pv[:, :FL].rearrange("p (h d) -> p h d", d=D)
                )
                nc.vector.memset(v3[:, :, D : D + 1], 1.0)

            # j-group ordering matches the transpose-DMA arrival order
            for j in range(4):
                for ft in range(FT):
                    qk_chunk(1, ft, j)
                if j == 0:
                    for ft in range(FT):
                        qk_chunk(0, ft, 0)
                for t in range(4 * j, 4 * j + 4):
                    v_tile(t)
                if j > 0:
                    for ft in range(FT):
                        qk_chunk(0, ft, j)

        # ---------------- Stage B + C ----------------
        with tc.tile_pool(name="s_psum", bufs=3, space="PSUM") as s_psum, \
             tc.tile_pool(name="o_psum", bufs=1, space="PSUM") as o_psum, \
             tc.tile_pool(name="espool", bufs=8) as espool, \
             tc.tile_pool(name="smalls", bufs=6) as smalls, \
             tc.tile_pool(name="osb", bufs=3) as osb, \
             tc.tile_pool(name="invdram", bufs=6, space="DRAM") as invdram:

            eng_ctr = [0]

            def exp_tile(es, s_t):
                g = eng_ctr[0]
                eng_ctr[0] += 1
                act = ((g + 1) * ACT_SHARE) // 16 > (g * ACT_SHARE) // 16
                if act:
                    nc.scalar.activation(es[:], s_t[:], Exp, scale=SCALE)
                else:
                    nc.vector.tensor_scalar(
                        es[:].bitcast(I16), s_t[:], K1S, K2S, Mult, Add
                    )

            def norm(hp, hh, c, posb):
                # reciprocal of the denominator row, broadcast back over 64
                # partitions via a DRAM bounce (SBUF sources can't have
                # step-0 partitions; DRAM can), multiply on GpSimd.
                sd = invdram.tile([1, 512], FP, tag="sd")
                nc.sync.dma_start(sd[:], posb[D : D + 1, :])
                s4 = smalls.tile([P, 4], FP, tag="s4")
                nc.sync.dma_start(s4[:], sd.rearrange("a (p f) -> (a p) f", p=P))
                inv4 = smalls.tile([P, 4], FP, tag="inv4")
                nc.vector.reciprocal(inv4[:], s4[:])
                invd = invdram.tile([1, 512], FP, tag="invd")
                nc.sync.dma_start(invd.rearrange("a (p f) -> (a p) f", p=P), inv4[:])
                inv64 = smalls.tile([D, 512], FP, tag="inv64")
                nc.sync.dma_start(inv64[:], invd[0:1, :].to_broadcast((D, 512)))
                nc.gpsimd.tensor_mul(
                    outT[hp][D * hh : D * (hh + 1), 512 * c : 512 * (c + 1)],
                    posb[0:D, :],
                    inv64[:],
                )

            def stage_c_tile(t):
                pf1 = o_psum.tile([P, 512], FP, tag="po0", name=f"pf1_{t}")
                pf2 = o_psum.tile([P, 512], FP, tag="po1", name=f"pf2_{t}")
                for ct in range(FT):
                    nc.tensor.matmul(
                        pf1[:],
                        outT[ct][:, P * t : P * (t + 1)],
                        wproj_sb[ct][:, 0:512],
                        start=(ct == 0),
                        stop=(ct == FT - 1),
                    )
                for ct in range(FT):
                    nc.tensor.matmul(
                        pf2[:, :256],
                        outT[ct][:, P * t : P * (t + 1)],
                        wproj_sb[ct][:, 512:E],
                        start=(ct == 0),
                        stop=(ct == FT - 1),
                    )
                ot = osb.tile([P, E], F16, tag="ot", name=f"ot{t}")
                nc.scalar.activation(ot[:, 0:512], pf1[:], Copy)
                nc.vector.tensor_copy(ot[:, 512:E], pf2[:, :256])
                nc.sync.dma_start(out16[P * t : P * (t + 1), :], ot[:])

            for c in range(NQC):
                for hp in range(FT):
                    po = {}
                    for hh in range(2):
                        po[hh] = o_psum.tile(
                            [P, 512], FP, tag=f"po{hh}", name=f"po{c}_{hp}_{hh}"
                        )
                    pend = []

                    def emit_av(p, hh, es):
                        h = 2 * hp + hh
                        for u in range(2):
                            nc.tensor.matmul(
                                po[hh][:DV, :],
                                vp[2 * p + u][:, DV * h : DV * (h + 1)],
                                es[:, 512 * u : 512 * (u + 1)],
                                start=(p == 0 and u == 0),
                                stop=(p == 7 and u == 1),
                            )

                    for p in range(8):
                        s_t = {}
                        for hh in range(2):
                            s_t[hh] = s_psum.tile(
                                [P, 1024], FP, tag="s", name=f"s{c}_{hp}_{p}_{hh}"
                            )
                        # row-tiled S^T: heads hh=0 (rows 0:64) and hh=1
                        # (rows 64:128) issue back-to-back and overlap
                        for u in range(2):
                            i = 2 * p + u
                            for hh in range(2):
                                nc.tensor.matmul(
                                    s_t[hh][:, 512 * u : 512 * (u + 1)],
                                    kT[hp][D * hh : D * (hh + 1), P * i : P * (i + 1)],
                                    qT[hp][D * hh : D * (hh + 1), 512 * c : 512 * (c + 1)],
                                    start=True,
                                    stop=True,
                                )
                        for hh in range(2):
                            es = espool.tile(
                                [P, 1024], F16, tag="es", name=f"es{c}_{hp}_{p}_{hh}"
                            )
                            exp_tile(es, s_t[hh])
                            pend.append((p, hh, es))
                        while pend and pend[0][0] <= p - LAG:
                            emit_av(*pend.pop(0))
                    for ent in pend:
                        emit_av(*ent)

                    for hh in range(2):
                        posb = smalls.tile(
                            [DV, 512], FP, tag="posb", bufs=4,
                            name=f"posb{c}_{hp}_{hh}",
                        )
                        nc.vector.tensor_copy(posb[:], po[hh][:DV, :])
                        norm(hp, hh, c, posb)

                # output projection for this query chunk (reuses po psum slots)
                for t in range(4 * c, 4 * c + 4):
                    stage_c_tile(t)

    _split_waits(nc)
    return nc


_CACHE = {}


def _get_nc():
    if "nc" not in _CACHE:
        _CACHE["nc"] = build()
    return _CACHE["nc"]


def make_in_maps(x, w_qkv, w_proj, b_proj):
    x = np.asarray(x, dtype=np.float32)
    w_qkv = np.asarray(w_qkv, np.float32)
    w_proj = np.asarray(w_proj, np.float32)
    in_maps = []
    for c in range(NCORES):
        b, half = c // 2, c % 2
        heads = range(HL * half, HL * half + HL)
        rows = (
            [E * 0 + D * h + d for h in heads for d in range(D)]
            + [E * 1 + D * h + d for h in heads for d in range(D)]
            + [E * 2 + D * h + d for h in heads for d in range(D)]
        )
        wqkvT_l = np.ascontiguousarray(w_qkv[rows, :].T).astype(np.float16)
        wprojT_l = np.ascontiguousarray(w_proj[:, rows[:FL]].T).astype(np.float16)
        in_maps.append(
            {
                "xb": np.ascontiguousarray(x[b]).astype(np.float16),
                "wqkvT": wqkvT_l,
                "wprojT": wprojT_l,
            }
        )
    return in_maps


def assemble(results, b_proj):
    b_proj = np.asarray(b_proj, np.float32)
    outp = np.empty((B, S, E), np.float32)
    for b in range(B):
        outp[b] = (
            results[2 * b]["out16"].astype(np.float32)
            + results[2 * b + 1]["out16"].astype(np.float32)
            + b_proj
        )
    return outp


def kernel(x, w_qkv, w_proj, b_proj):
    nc = _get_nc()
    in_maps = make_in_maps(x, w_qkv, w_proj, b_proj)
    res = run_bass_kernel_spmd(nc, in_maps, core_ids=list(range(NCORES)))
    return assemble(res.results, b_proj)


# revision 11
# speedup vs baseline: 1.2581x; 1.2581x over previous
"""Multi-head attention (B=4, S=2048, E=768, H=12) on 8 trn2 NeuronCores.

Sharding: tensor-parallel over heads x data-parallel over batch. Core c
handles batch b=c//2 and heads 6*(c%2)..6*(c%2)+5 (all 2048 queries). Each
core emits a partial output projection (its 6 heads' contribution); the two
cores of a batch pair are summed on the host during unsharding, where the
bias is also added (host-side add is part of the same unshard pass that
already sums the core pairs).

v3 design (from the v1 trace: PE busy 285.6us/322us = the bottleneck, with
Scalar exp at 226.6us right behind; v2 trace: 43us startup stall on
transpose-DMAs, ~8us PE head-of-line stalls per q-chunk boundary, and
(64,128)-tile S^T matmuls forcing PE tiling-mode drains against the
128-contract AV matmuls):
  - The host supplies x already transposed as fp16 [E, S]: x^T costs the
    device nothing (v1 spent 26us PE + 15us ACT on transposes/casts; v2's
    xbar transpose-DMAs ran at ~25GB/s/queue and stalled the PE 43us).
  - kT is stored zero-PADDED per head: kTp[h] is [128, S] with the head's
    64 d-rows as data and the other 64 rows zero. S^T then contracts the
    full 128 partitions (0 * junk = 0 keeps it exact, so qT stays shared
    and unpadded) and every matmul in the kernel is a standard
    (128,128)-tile op -- no PE tiling-mode switches, no drains.
  - The exp stream (25.2M logits/core) is split across two engines: ACT
    runs exact exp (9/16 of tiles); DVE runs a Schraudolph bit-trick exp
    (7/16): one tensor_scalar (mult,add) producing int16 fp16-bits
    (numpy sim of the full pipeline: rel err 1.0e-2 vs the 2e-2 budget).
    The softmax denominator is the V ones-column, so numerator and
    denominator use identical approximated weights and the residual error
    is a mild reweighting.
  - Stage B runs in 12 blocks (4 query-chunks x 3 head-pairs) of 8
    k-tile-pairs. PSUM: 3-deep [128,1024] S ring (6 banks) + 2 AV
    accumulator banks = 8 exactly. AV accumulates all 16 k-tiles per
    (head, q-chunk) with DV=65 (ones column) at LAG>=2 behind exp.
  - Softmax normalization multiplies run on the otherwise idle GpSimd
    engine; the reciprocal keeps v1's DRAM-bounce partition broadcast.
  - Output projection for query chunk c is emitted after the first head
    block of chunk c+1, so its wait on the norm's DMA-bounce chain never
    head-of-line-blocks the next chunk's S^T matmuls in the PE FIFO.
    Results leave as fp16 and are upcast host-side.

Environment workarounds (this walrus build): sync-waits are split one per
instruction onto NoOps (_split_waits, _TC).
"""

import math

import numpy as np

import concourse.bass as bass
import concourse.tile as tile
from concourse import mybir
from concourse.bass_utils import run_bass_kernel_spmd
from concourse.tile import ScopedClock

B, S, E, H, D = 4, 2048, 768, 12, 64
NCORES = 8
HL = 6               # heads per core
FL = HL * D          # 384 local feature dim
SCALE = D ** -0.5
FP = mybir.dt.float32
F16 = mybir.dt.float16
I16 = mybir.dt.int16
P = 128

ET = E // P          # 6 e-chunks of 128
FT = FL // P         # 3 local f-tiles of 128
NKT = S // P         # 16 k-tiles of 128
NQC = S // 512       # 4 query chunks of 512
NST = S // P         # 16 s-tiles
DV = D + 1           # 65: V plus ones column

# Schraudolph fp16-bit exp: bits = round(SCALE*s*1024/ln2 + 15360 + SIGMA)
SIGMA = -60.0
K1S = SCALE * 1024.0 / math.log(2.0)
K2S = 15360.0 + SIGMA
# ACT gets 9 of every 16 exp tiles (ACT ~1147ns vs DVE ~1192ns + DVE extras)
ACT_SHARE = 9

LAG = 2              # AV trails exp by this many k-tile pairs


class _TC(tile.TileContext):
    """TileContext with the end-of-kernel drain's sem waits split one per
    instruction (this walrus build's CTRL_NO_STRUCT encoding holds only one
    sync wait; the stock drain carries one wait per outstanding proc)."""

    def _drain_and_barrier(self, tick_clock, wait_clock):
        probe = self.nc.sync.nop()
        wait_clock.add_sem_waits(
            probe.ins, ScopedClock({None: tick_clock.global_clock})
        )
        si = probe.ins.sync_info
        waits = list(si.on_wait) if si is not None else []
        if len(waits) > 1:
            si.on_wait = waits[:1]
            for w in waits[1:]:
                n = self.nc.sync.nop()
                n.ins.sync_info = type(si)(on_wait=[w], on_update=[])
        self.nc.sync.drain()
        self.nc.all_engine_barrier()
        popped = self.nc._tile_sem_poison_stack.pop()
        assert popped is self._sem_poison
        self.nc.clear_and_free_semaphores(list(self.sems.allocated().values()))
        self.nc.all_engine_barrier()


def _split_waits(nc):
    """This walrus build accepts at most one sync-wait per TPB instruction
    (two on EventSemaphore). Tile emits up to 2-3. Hoist the extras onto
    same-engine NoOps inserted immediately before the instruction."""
    ctr = [0]
    for f in nc.m.functions:
        for bb in f.blocks:
            out = []
            changed = False
            for inst in bb.instructions:
                si = getattr(inst, "sync_info", None)
                if si is not None and si.on_wait:
                    cap = 2 if isinstance(inst, mybir.InstEventSemaphore) else 1
                    waits = list(si.on_wait)
                    if len(waits) > cap:
                        changed = True
                        for w in waits[:-cap]:
                            ctr[0] += 1
                            out.append(
                                mybir.InstNoOp(
                                    name=f"WSPLIT-{ctr[0]}",
                                    engine=inst.engine,
                                    ins=[],
                                    outs=[],
                                    sync_info=mybir.SyncInfo(
                                        on_wait=[w], on_update=[]
                                    ),
                                    bass_nofuse=True,
                                )
                            )
                        si.on_wait = waits[-cap:]
                        inst.sync_info = si
                out.append(inst)
            if changed:
                bb.instructions = out


def build():
    nc = bass.Bass()
    xbTd = nc.dram_tensor("xbTd", [E, S], F16, kind="ExternalInput")
    wqkvT = nc.dram_tensor("wqkvT", [E, 3 * FL], F16, kind="ExternalInput")
    wprojT = nc.dram_tensor("wprojT", [FL, E], F16, kind="ExternalInput")
    out16 = nc.dram_tensor("out16", [S, E], F16, kind="ExternalOutput")

    Exp = mybir.ActivationFunctionType.Exp
    Copy = mybir.ActivationFunctionType.Copy
    Mult = mybir.AluOpType.mult
    Add = mybir.AluOpType.add

    from contextlib import ExitStack

    with _TC(nc) as tc, ExitStack() as stack:
        consts = stack.enter_context(tc.tile_pool(name="consts", bufs=1))
        persist = stack.enter_context(tc.tile_pool(name="persist", bufs=1))

        wqkv_sb = [
            consts.tile([P, 3 * FL], F16, tag=f"wqkv{c}", name=f"wqkv{c}")
            for c in range(ET)
        ]
        wproj_sb = [
            consts.tile([P, E], F16, tag=f"wproj{c}", name=f"wproj{c}")
            for c in range(FT)
        ]
        xbT = [
            persist.tile([P, S], F16, tag=f"xbT{c}", name=f"xbT{c}")
            for c in range(ET)
        ]
        qT = [persist.tile([P, S], F16, tag=f"qT{t}", name=f"qT{t}") for t in range(FT)]
        # kTp[h]: zero-padded K^T per head -- the head's 64 d-rows at their
        # native partition offset (64*(h%2)), other 64 rows stay zero
        kTp = [persist.tile([P, S], F16, tag=f"kTp{h}", name=f"kTp{h}") for h in range(HL)]
        vp = [persist.tile([P, HL * DV], F16, tag=f"vp{t}", name=f"vp{t}") for t in range(NST)]
        outT = [persist.tile([P, S], F16, tag=f"outT{t}", name=f"outT{t}") for t in range(FT)]

        # ---------------- input DMAs ----------------
        for c in range(ET):
            nc.scalar.dma_start(wqkv_sb[c][:], wqkvT[P * c : P * (c + 1), :])
        for c in range(FT):
            nc.scalar.dma_start(wproj_sb[c][:], wprojT[P * c : P * (c + 1), :])
        # x^T comes pre-transposed from the host; plain fast DMAs
        for c in range(ET):
            nc.sync.dma_start(xbT[c][:], xbTd[P * c : P * (c + 1), :])
        # zero the padded-K tiles once (only the data rows get overwritten)
        for h in range(HL):
            nc.gpsimd.memset(kTp[h][:], 0.0)

        # ---------------- Stage A: QKV projections ----------------
        with tc.tile_pool(name="mm_psum", bufs=3, space="PSUM") as mm_psum:

            def qk_chunk(which, ft, j):
                pq = mm_psum.tile([P, 512], FP, tag="mm", name=f"pq{which}_{ft}_{j}")
                for c in range(ET):
                    nc.tensor.matmul(
                        pq[:],
                        wqkv_sb[c][:, FL * which + P * ft : FL * which + P * (ft + 1)],
                        xbT[c][:, 512 * j : 512 * (j + 1)],
                        start=(c == 0),
                        stop=(c == ET - 1),
                    )
                cols = slice(512 * j, 512 * (j + 1))
                if which == 0:
                    nc.vector.tensor_copy(qT[ft][:, cols], pq[:])
                else:
                    # K lands zero-padded per head at its native partition
                    # offset; copies run on the stage-A-idle ACT engine
                    Copy = mybir.ActivationFunctionType.Copy
                    for hh in range(2):
                        nc.scalar.activation(
                            kTp[2 * ft + hh][D * hh : D * (hh + 1), cols],
                            pq[D * hh : D * (hh + 1), :],
                            Copy,
                        )

            def v_tile(t):
                pv = mm_psum.tile([P, 512], FP, tag="mm", name=f"pv{t}")
                for c in range(ET):
                    nc.tensor.matmul(
                        pv[:, :FL],
                        xbT[c][:, P * t : P * (t + 1)],
                        wqkv_sb[c][:, 2 * FL : 3 * FL],
                        start=(c == 0),
                        stop=(c == ET - 1),
                    )
                v3 = vp[t].rearrange("p (h d) -> p h d", d=DV)
                nc.vector.tensor_copy(
                    v3[:, :, 0:D], pv[:, :FL].rearrange("p (h d) -> p h d", d=D)
                )
                nc.vector.memset(v3[:, :, D : D + 1], 1.0)

            # j-group ordering matches the transpose-DMA arrival order
            for j in range(4):
                for ft in range(FT):
                    qk_chunk(1, ft, j)
                if j == 0:
                    for ft in range(FT):
                        qk_chunk(0, ft, 0)
                for t in range(4 * j, 4 * j + 4):
                    v_tile(t)
                if j > 0:
                    for ft in range(FT):
                        qk_chunk(0, ft, j)

        # ---------------- Stage B + C ----------------
        with tc.tile_pool(name="s_psum", bufs=3, space="PSUM") as s_psum, \
             tc.tile_pool(name="o_psum", bufs=1, space="PSUM") as o_psum, \
             tc.tile_pool(name="espool", bufs=8) as espool, \
             tc.tile_pool(name="smalls", bufs=6) as smalls, \
             tc.tile_pool(name="osb", bufs=3) as osb, \
             tc.tile_pool(name="invdram", bufs=6, space="DRAM") as invdram:

            eng_ctr = [0]

            def exp_tile(es, s_t):
                g = eng_ctr[0]
                eng_ctr[0] += 1
                act = ((g + 1) * ACT_SHARE) // 16 > (g * ACT_SHARE) // 16
                if act:
                    nc.scalar.activation(es[:], s_t[:], Exp, scale=SCALE)
                else:
                    nc.vector.tensor_scalar(
                        es[:].bitcast(I16), s_t[:], K1S, K2S, Mult, Add
                    )

            def norm(hp, hh, c, posb):
                # reciprocal of the denominator row, broadcast back over 64
                # partitions via a DRAM bounce (SBUF sources can't have
                # step-0 partitions; DRAM can), multiply on GpSimd.
                sd = invdram.tile([1, 512], FP, tag="sd")
                nc.sync.dma_start(sd[:], posb[D : D + 1, :])
                s4 = smalls.tile([P, 4], FP, tag="s4")
                nc.sync.dma_start(s4[:], sd.rearrange("a (p f) -> (a p) f", p=P))
                inv4 = smalls.tile([P, 4], FP, tag="inv4")
                nc.vector.reciprocal(inv4[:], s4[:])
                invd = invdram.tile([1, 512], FP, tag="invd")
                nc.sync.dma_start(invd.rearrange("a (p f) -> (a p) f", p=P), inv4[:])
                inv64 = smalls.tile([D, 512], FP, tag="inv64")
                nc.sync.dma_start(inv64[:], invd[0:1, :].to_broadcast((D, 512)))
                nc.gpsimd.tensor_mul(
                    outT[hp][D * hh : D * (hh + 1), 512 * c : 512 * (c + 1)],
                    posb[0:D, :],
                    inv64[:],
                )

            def stage_c_tile(t):
                pf1 = o_psum.tile([P, 512], FP, tag="po0", name=f"pf1_{t}")
                pf2 = o_psum.tile([P, 512], FP, tag="po1", name=f"pf2_{t}")
                for ct in range(FT):
                    nc.tensor.matmul(
                        pf1[:],
                        outT[ct][:, P * t : P * (t + 1)],
                        wproj_sb[ct][:, 0:512],
                        start=(ct == 0),
                        stop=(ct == FT - 1),
                    )
                for ct in range(FT):
                    nc.tensor.matmul(
                        pf2[:, :256],
                        outT[ct][:, P * t : P * (t + 1)],
                        wproj_sb[ct][:, 512:E],
                        start=(ct == 0),
                        stop=(ct == FT - 1),
                    )
                ot = osb.tile([P, E], F16, tag="ot", name=f"ot{t}")
                nc.scalar.activation(ot[:, 0:512], pf1[:], Copy)
                nc.vector.tensor_copy(ot[:, 512:E], pf2[:, :256])
                nc.sync.dma_start(out16[P * t : P * (t + 1), :], ot[:])

            for c in range(NQC):
                for hp in range(FT):
                    po = {}
                    for hh in range(2):
                        po[hh] = o_psum.tile(
                            [P, 512], FP, tag=f"po{hh}", name=f"po{c}_{hp}_{hh}"
                        )
                    pend = []

                    def emit_av(p, hh, es):
                        h = 2 * hp + hh
                        for u in range(2):
                            nc.tensor.matmul(
                                po[hh][:DV, :],
                                vp[2 * p + u][:, DV * h : DV * (h + 1)],
                                es[:, 512 * u : 512 * (u + 1)],
                                start=(p == 0 and u == 0),
                                stop=(p == 7 and u == 1),
                            )

                    for p in range(8):
                        s_t = {}
                        for hh in range(2):
                            s_t[hh] = s_psum.tile(
                                [P, 1024], FP, tag="s", name=f"s{c}_{hp}_{p}_{hh}"
                            )
                        # S^T at full 128 contract: kTp's zero rows null out
                        # the other head's qT rows, so qT is shared unsliced
                        for u in range(2):
                            i = 2 * p + u
                            for hh in range(2):
                                nc.tensor.matmul(
                                    s_t[hh][:, 512 * u : 512 * (u + 1)],
                                    kTp[2 * hp + hh][:, P * i : P * (i + 1)],
                                    qT[hp][:, 512 * c : 512 * (c + 1)],
                                    start=True,
                                    stop=True,
                                )
                        for hh in range(2):
                            es = espool.tile(
                                [P, 1024], F16, tag="es", name=f"es{c}_{hp}_{p}_{hh}"
                            )
                            exp_tile(es, s_t[hh])
                            pend.append((p, hh, es))
                        while pend and pend[0][0] <= p - LAG:
                            emit_av(*pend.pop(0))
                    for ent in pend:
                        emit_av(*ent)

                    for hh in range(2):
                        posb = smalls.tile(
                            [DV, 512], FP, tag="posb", bufs=4,
                            name=f"posb{c}_{hp}_{hh}",
                        )
                        nc.vector.tensor_copy(posb[:], po[hh][:DV, :])
                        norm(hp, hh, c, posb)

                    # output projection for the PREVIOUS query chunk goes
                    # here (after this chunk's first head block) so its wait
                    # on the norm bounce chain is long satisfied and never
                    # head-of-line-blocks S^T matmuls in the PE FIFO
                    if hp == 0 and c > 0:
                        for t in range(4 * (c - 1), 4 * c):
                            stage_c_tile(t)
            for t in range(4 * (NQC - 1), 4 * NQC):
                stage_c_tile(t)

    _split_waits(nc)
    return nc


_CACHE = {}


def _get_nc():
    if "nc" not in _CACHE:
        _CACHE["nc"] = build()
    return _CACHE["nc"]


def make_in_maps(x, w_qkv, w_proj, b_proj):
    x = np.asarray(x, dtype=np.float32)
    w_qkv = np.asarray(w_qkv, np.float32)
    w_proj = np.asarray(w_proj, np.float32)
    in_maps = []
    for c in range(NCORES):
        b, half = c // 2, c % 2
        heads = range(HL * half, HL * half + HL)
        rows = (
            [E * 0 + D * h + d for h in heads for d in range(D)]
            + [E * 1 + D * h + d for h in heads for d in range(D)]
            + [E * 2 + D * h + d for h in heads for d in range(D)]
        )
        wqkvT_l = np.ascontiguousarray(w_qkv[rows, :].T).astype(np.float16)
        wprojT_l = np.ascontiguousarray(w_proj[:, rows[:FL]].T).astype(np.float16)
        in_maps.append(
            {
                "xbTd": np.ascontiguousarray(x[b].T.astype(np.float16)),
                "wqkvT": wqkvT_l,
                "wprojT": wprojT_l,
            }
        )
    return in_maps


def assemble(results, b_proj):
    b_proj = np.asarray(b_proj, np.float32)
    outp = np.empty((B, S, E), np.float32)
    for b in range(B):
        outp[b] = (
            results[2 * b]["out16"].astype(np.float32)
            + results[2 * b + 1]["out16"].astype(np.float32)
            + b_proj
        )
    return outp


def kernel(x, w_qkv, w_proj, b_proj):
    nc = _get_nc()
    in_maps = make_in_maps(x, w_qkv, w_proj, b_proj)
    res = run_bass_kernel_spmd(nc, in_maps, core_ids=list(range(NCORES)))
    return assemble(res.results, b_proj)
